# revision 52
# baseline (speedup 1.0000x reference)
"""Trainium2 Bass kernel for a 4-layer GPT-style transformer (B=2, S=1024,
D=512, H=8, DFF=2048, V=50257) on 8 NeuronCores.

Sharding: 2 batch groups x 4-way sequence-parallel (tokens interleaved
stride 4); K/V all-gathered per layer within each group, final hidden
states all-gathered 8-way for a vocab-sharded LM head.  Key optimizations:
bf16 logits, softmax denominator fused into the AV matmul via a ones
column in V, K/V projected before Q so the AllGather overlaps Q, per-rank
contiguous KV blocks (minimal DMA count), capped weight-prefetch priority
hoists, v-bias folded through Wo on the host."""

import numpy as np
import ml_dtypes

import concourse.bass as bass
import concourse.mybir as mybir
import concourse.tile as tile
from concourse import bacc
from concourse.bass_utils import run_bass_kernel_spmd
from concourse.masks import make_identity

AF = mybir.ActivationFunctionType
Alu = mybir.AluOpType
f32 = mybir.dt.float32
bf16 = mybir.dt.bfloat16

V, D, H, DK, DFF, L, B, S = 50257, 512, 8, 64, 2048, 4, 2, 1024
NC, P = 8, 128
EPS = 1e-5
TOK = 256
NT = TOK // P                # 2
KD = D // P                  # 4
KF = DFF // P                # 16
VS_PAD = 6400
VT_FULL = VS_PAD // P        # 50
VS = [6283] * 7 + [V - 7 * 6283]

_bf = lambda a: np.ascontiguousarray(np.asarray(a).astype(ml_dtypes.bfloat16))
_f32 = lambda a: np.ascontiguousarray(np.asarray(a, dtype=np.float32))


def _wload_ap(dram, kdim, n):
    """DRAM [kdim*P, n] viewed as dest-tile order [P, kdim, n]"""
    return bass.AP(tensor=dram, offset=0,
                   ap=[[n, P], [P * n, kdim], [1, n]])


def build(n_layers=L, vt=VT_FULL, debug=False, sim=False):
    nc = bacc.Bacc("TRN2", target_bir_lowering=False, debug=False, num_devices=NC)

    x0_in = nc.dram_tensor("x0", [TOK, D], f32, kind="ExternalInput")
    masks_in = nc.dram_tensor("masks", [4, P, P], bf16, kind="ExternalInput")
    Ws = []
    for l in range(n_layers):
        Ws.append({k: nc.dram_tensor(f"{k}{l}", shp, dt, kind="ExternalInput")
                   for k, shp, dt in [
                       ("wq", [D, D], bf16), ("wk", [D, D], bf16),
                       ("wv", [D, D], bf16), ("wo", [D, D], bf16),
                       ("w1", [D, DFF], bf16), ("w2", [DFF, D], bf16),
                       ("bvec", [P, 28], f32),      # bq|bk|bv|b1 per-partition
                       ("bod", [2, D], f32)]})      # bo | b2 rows (bcast on-chip)
    whead = nc.dram_tensor("whead", [D, vt * P], bf16, kind="ExternalInput")
    bhead = nc.dram_tensor("bhead", [vt * P], f32, kind="ExternalInput")
    logits_out = nc.dram_tensor("logitsT", [vt * P, B * S], bf16, kind="ExternalOutput")
    xdbg = (nc.dram_tensor("xdbg", [n_layers, TOK, D], f32, kind="ExternalOutput")
            if debug else None)

    GROUPS_BATCH = [[0, 1, 2, 3], [4, 5, 6, 7]]
    GROUPS_ALL = [list(range(NC))]
    KSZ = D * TOK                 # KT_own elems per rank
    VSZ = TOK * H * (DK + 1)      # V_own(+ones) elems per rank
    ags = []
    for l in range(n_layers):
        aik = nc.dram_tensor(f"agink{l}", [KSZ], bf16)
        aok = nc.dram_tensor(f"agoutk{l}", [4 * KSZ], bf16)
        aiv = nc.dram_tensor(f"aginv{l}", [VSZ], bf16)
        aov = nc.dram_tensor(f"agoutv{l}", [4 * VSZ], bf16)
        ags.append((aik, aok, aiv, aov))
    af_i = nc.dram_tensor("aginF", [D, TOK], bf16)
    af_o = nc.dram_tensor("agoutF", [NC * D, TOK], bf16, addr_space="Shared")

    import contextlib
    with tile.TileContext(nc) as tc, contextlib.ExitStack() as ctx:
        const = ctx.enter_context(tc.tile_pool(name="const", bufs=1))
        xp = ctx.enter_context(tc.tile_pool(name="xp", bufs=2))
        wp = ctx.enter_context(tc.tile_pool(name="wp", bufs=1))
        whp = ctx.enter_context(tc.tile_pool(name="whp", bufs=1))
        act = ctx.enter_context(tc.tile_pool(name="act", bufs=2))
        atn = ctx.enter_context(tc.tile_pool(name="atn", bufs=2))
        sm = ctx.enter_context(tc.tile_pool(name="sm", bufs=2))
        ps = ctx.enter_context(tc.tile_pool(name="ps", bufs=3, space="PSUM"))
        psu = ctx.enter_context(tc.tile_pool(name="psu", bufs=2, space="PSUM"))
        pss = ctx.enter_context(tc.tile_pool(name="pss", bufs=2, space="PSUM"))

        ident = const.tile([P, P], bf16)
        make_identity(nc, ident)
        ones_col = const.tile([P, 1], bf16)
        nc.vector.memset(ones_col, 1.0)
        eps_t = const.tile([P, 1], f32)
        nc.vector.memset(eps_t, EPS)
        neg1_t = const.tile([P, 1], f32)
        nc.vector.memset(neg1_t, -1.0)
        masks = const.tile([P, 4, P], bf16)
        x_t = [xp.tile([P, D], f32, tag=f"x{t}", name=f"x_{t}") for t in range(NT)]
        with tc.high_priority():
            for t in range(NT):
                nc.sync.dma_start(out=x_t[t], in_=x0_in[t * P:(t + 1) * P, :])
            nc.sync.dma_start(out=masks, in_=bass.AP(
                tensor=masks_in, offset=0, ap=[[P, P], [P * P, 4], [1, P]]))

        def ln_tile(src, tag, t):
            stats = sm.tile([P, 6], f32, tag="stats")
            nc.vector.bn_stats(stats, src)
            mv = sm.tile([P, 2], f32, tag="mv")
            nc.vector.bn_aggr(mv, stats)
            sd = sm.tile([P, 1], f32, tag="sd")
            nc.scalar.activation(sd, mv[:, 1:2], AF.Sqrt, bias=eps_t, scale=1.0)
            nc.vector.reciprocal(sd, sd)
            h = act.tile([P, D], bf16, tag=f"{tag}{t}")
            nc.vector.tensor_scalar(
                out=h, in0=src, scalar1=mv[:, 0:1], scalar2=sd,
                op0=Alu.subtract, op1=Alu.mult)
            return h

        def layernorm(src_tiles, tag):
            return [ln_tile(src_tiles[t], tag, t) for t in range(NT)]

        def transpose_tile(hT, h, t):
            for d in range(KD):
                pt = ps.tile([P, P], bf16, tag="mm", bufs=3)
                nc.tensor.transpose(pt, h[:, d * P:(d + 1) * P], ident)
                if d % 2 == 0:
                    nc.vector.tensor_copy(hT[:, d, t * P:(t + 1) * P], pt)
                else:
                    nc.scalar.copy(hT[:, d, t * P:(t + 1) * P], pt)

        def ln_transpose(src_tiles, tag, ttag, anchors=None):
            hT = act.tile([P, KD, TOK], bf16, tag=ttag)
            for t in range(NT):
                a = anchors[t] if anchors else None
                if a is not None:
                    with tc.high_priority(
                            offset=max(tc.cur_priority - a, 0)):
                        h = ln_tile(src_tiles[t], tag, t)
                else:
                    h = ln_tile(src_tiles[t], tag, t)
                transpose_tile(hT, h, t)
            return hT

        def resid_ln_transpose(pzs, xbs, tag, ttag, dbg=None):
            # deferred residual: tile t's add+LN chain is issued before tile
            # t+1's, so the DVE FIFO runs t0's stats while PE still streams
            # t1's matmuls, and t0's transposes start ~2us earlier
            hT = act.tile([P, KD, TOK], bf16, tag=ttag)
            for t in range(NT):
                xn = xp.tile([P, D], f32, tag=f"x{t}")
                nc.vector.tensor_add(xn, pzs[t], xbs[t])
                x_t[t] = xn
                if dbg is not None:
                    nc.sync.dma_start(out=dbg[t], in_=xn)
                h = ln_tile(xn, tag, t)
                transpose_tile(hT, h, t)
            return hT

        def transpose_own(h_tiles, tag):
            # t-outer so downstream per-t consumers can start after tile 0
            hT = act.tile([P, KD, TOK], bf16, tag=tag)
            for t in range(NT):
                transpose_tile(hT, h_tiles[t], t)
            return hT

        def preload_table(func):
            # tiny dummy activation issued while Act is idle: pulls the next
            # activation-table load off the critical path
            if preload_table.off:
                return
            dummy = sm.tile([P, 1], f32, tag="dummy")
            nc.scalar.activation(dummy, eps_t, func, bias=0.0, scale=1.0)
        preload_table.off = bool(__import__('os').environ.get('NO_PRELOAD'))

        def load_w(dram, kdim, ndim, tag, offset=180, bufs=1, swdge=False):
            off = min(offset, max(tc.cur_priority - 8, 0))
            with tc.high_priority(offset=off):
                wt = wp.tile([P, kdim, ndim], bf16, tag=tag, name=tag, bufs=bufs)
                # SWDGE (Pool) path skips the serial HWDGE gate; safe only
                # before the first collective is queued on the Pool engine
                eng = nc.gpsimd if swdge else nc.sync
                eng.dma_start(out=wt, in_=_wload_ap(dram, kdim, ndim))
            return wt

        pending = None
        for l in range(n_layers):
            W = Ws[l]
            aik, aok, aiv, aov = ags[l]

            # ---- LN1, transpose ----
            if pending is None:
                hT_own = ln_transpose(x_t, "h1_", "hTown")
            else:
                hT_own = resid_ln_transpose(pending[0], pending[1],
                                            "h1_", "hTown", dbg=pending[2])

            wq_sb = load_w(W["wq"], KD, D, "wq", bufs=2)
            wk_sb = load_w(W["wk"], KD, D, "wk", bufs=2)
            wv_sb = load_w(W["wv"], KD, D, "wv", bufs=2)
            bv_sb = sm.tile([P, 28], f32, tag="bvec")
            nc.sync.dma_start(out=bv_sb, in_=W["bvec"][:, :])
            bq_t = bv_sb[:, 0:4]; bk_t = bv_sb[:, 4:8]
            b1_t = bv_sb[:, 12:28]

            # ---- K, V for OWN tokens first (feeds the AllGather) ----
            # one contiguous [P, KVC] block: K^T cols then V rows, V rows
            # augmented with a per-head ones column (fused softmax denom)
            VW = H * (DK + 1)
            KVC = KD * TOK + NT * VW
            kv_own = atn.tile([P, KVC], bf16, tag="kvown", bufs=1)
            kT_own = kv_own[:, 0:KD * TOK].rearrange(
                "p (k t) -> p k t", k=KD)
            v_own = kv_own[:, KD * TOK:KVC].rearrange(
                "p (t h d) -> p t h d", t=NT, h=H)
            # K-proj per token-tile so tile-0 matmuls start before tile-1's
            # transpose lands
            for t in range(NT):
                for m in range(KD):
                    pk = ps.tile([P, P], f32, tag="mm")
                    for k in range(KD):
                        nc.tensor.matmul(pk, wk_sb[:, k, m * P:(m + 1) * P],
                                         hT_own[:, k, t * P:(t + 1) * P],
                                         start=(k == 0), stop=(k == KD - 1))
                    dst = kT_own[:, m, t * P:(t + 1) * P]
                    if m % 2 == 0:
                        nc.scalar.activation(dst, pk, AF.Identity,
                                             bias=bk_t[:, m:m + 1], scale=1.0)
                    else:
                        nc.vector.tensor_scalar_add(out=dst, in0=pk,
                                                    scalar1=bk_t[:, m:m + 1])

            # ---- AllGather K right away (V's gather launches separately
            # once V is projected, hiding under the K-gated score matmuls) ----
            with tc.high_priority(offset=16):
                nc.sync.dma_start(
                    out=bass.AP(tensor=aik, offset=0, ap=[[KSZ // P, P], [1, KD * TOK]]),
                    in_=kv_own[:, 0:KD * TOK])
                if sim:
                    for jp in range(4):
                        nc.sync.dma_start(out=aok[jp * KSZ:(jp + 1) * KSZ],
                                          in_=aik[:])
                else:
                    nc.gpsimd.collective_compute(
                        "AllGather", Alu.bypass, replica_groups=GROUPS_BATCH,
                        ins=[aik.ap().opt()], outs=[aok.ap().opt()])

            nc.vector.memset(v_own[:, :, :, DK:DK + 1], 1.0)
            for t in range(NT):
                pv = ps.tile([P, H, DK], f32, tag="mm")
                for k in range(KD):
                    nc.tensor.matmul(pv, hT_own[:, k, t * P:(t + 1) * P],
                                     wv_sb[:, k, :], start=(k == 0),
                                     stop=(k == KD - 1))
                if t % 2 == 0:
                    nc.vector.tensor_copy(v_own[:, t, :, 0:DK], pv)
                else:
                    nc.scalar.copy(v_own[:, t, :, 0:DK], pv)

            # ---- AllGather V within the batch group ----
            with tc.high_priority(offset=16):
                nc.sync.dma_start(
                    out=bass.AP(tensor=aiv, offset=0,
                                ap=[[VSZ // P, P], [1, NT * VW]]),
                    in_=kv_own[:, KD * TOK:KVC])
                if sim:
                    for jp in range(4):
                        nc.sync.dma_start(out=aov[jp * VSZ:(jp + 1) * VSZ],
                                          in_=aiv[:])
                else:
                    nc.gpsimd.collective_compute(
                        "AllGather", Alu.bypass, replica_groups=GROUPS_BATCH,
                        ins=[aiv.ap().opt()], outs=[aov.ap().opt()])

            # ---- Q for OWN tokens (overlaps the AllGather flight) ----
            qT = atn.tile([P, KD, TOK], bf16, tag="qT")
            for m in range(KD):
                pq = ps.tile([P, TOK], f32, tag="mm")
                for k in range(KD):
                    nc.tensor.matmul(pq, wq_sb[:, k, m * P:(m + 1) * P],
                                     hT_own[:, k, :],
                                     start=(k == 0), stop=(k == KD - 1))
                nc.scalar.activation(qT[:, m, :], pq, AF.Identity,
                                     bias=bq_t[:, m:m + 1], scale=1.0)
            preload_table(AF.Exp)

            kvj = [atn.tile([P, KVC], bf16, tag=f"kv{jp}", bufs=1,
                            name=f"kvj{jp}") for jp in range(4)]
            with tc.high_priority(offset=16):
                for jp in range(4):
                    nc.sync.dma_start(
                        out=kvj[jp][:, 0:KD * TOK],
                        in_=bass.AP(tensor=aok, offset=jp * KSZ,
                                    ap=[[KSZ // P, P], [1, KD * TOK]]))
                for jp in range(4):
                    nc.sync.dma_start(
                        out=kvj[jp][:, KD * TOK:KVC],
                        in_=bass.AP(tensor=aov, offset=jp * VSZ,
                                    ap=[[VSZ // P, P], [1, NT * VW]]))

            # ---- attention per head ----
            kTg = [kvj[jp][:, 0:KD * TOK].rearrange("p (k t) -> p k t", k=KD)
                   for jp in range(4)]
            vg = [kvj[jp][:, KD * TOK:KVC].rearrange(
                "p (t h d) -> p t h d", t=NT, h=H) for jp in range(4)]
            oT = atn.tile([P, KD, TOK], bf16, tag="oT")

            def head_scores(h):
                mt, bp = h // 2, 64 * (h % 2)
                kh = lambda col0, n: kTg[col0 // TOK][
                    bp:bp + DK, mt, (col0 % TOK):(col0 % TOK) + n]
                qh = qT[bp:bp + DK, mt, :]
                pT0 = atn.tile([P, 4, TOK], bf16, tag="pT0")
                pT1 = atn.tile([P, 4, P], bf16, tag="pT1")
                for pr in range(2):
                    sc = pss.tile([P, 2 * TOK], f32, tag="sc", bufs=3)
                    for i in range(2):
                        jp = 2 * pr + i
                        nc.tensor.matmul(sc[:, i * TOK:(i + 1) * TOK],
                                         kh(256 * jp, P), qh,
                                         start=True, stop=True,
                                         skip_group_check=True)
                    nc.scalar.activation(pT0[:, 2 * pr:2 * pr + 2, :], sc, AF.Exp)
                    for i in range(2):
                        jp = 2 * pr + i
                        nc.vector.tensor_mul(pT0[:, jp, 0:P], pT0[:, jp, 0:P],
                                             masks[:, jp, :])
                for pr in range(2):
                    sc1 = pss.tile([P, TOK], f32, tag="sc", bufs=3)
                    for i in range(2):
                        jp = 2 * pr + i
                        nc.tensor.matmul(sc1[:, i * P:(i + 1) * P],
                                         kh(256 * jp + P, P), qh[:, P:TOK],
                                         start=True, stop=True,
                                         skip_group_check=True)
                    nc.scalar.activation(pT1[:, 2 * pr:2 * pr + 2, :], sc1, AF.Exp)
                    for i in range(2):
                        jp = 2 * pr + i
                        nc.vector.tensor_mul(pT1[:, jp, :], pT1[:, jp, :],
                                             masks[:, jp, :])
                return pT0, pT1

            def head_av(h, pT0, pT1):
                mt, bp = h // 2, 64 * (h % 2)
                # u^T accumulation; row DK = softmax denominator (ones col)
                pu = psu.tile([DK + 1, TOK], f32, tag="pu")
                vh = lambda i: vg[i // 2][:, i % 2, h, :]
                for jp in range(4):
                    nc.tensor.matmul(pu, vh(2 * jp), pT0[:, jp, :],
                                     start=(jp == 0), stop=False,
                                     skip_group_check=True)
                for jp in range(4):
                    nc.tensor.matmul(pu[:, P:TOK], vh(2 * jp + 1), pT1[:, jp, :],
                                     start=False, stop=(jp == 3),
                                     skip_group_check=True)
                rec = sm.tile([1, TOK], f32, tag="rec", bufs=2)
                nc.vector.reciprocal(rec, pu[DK:DK + 1, :])
                recb = sm.tile([DK, TOK], f32, tag="recb", bufs=2)
                nc.gpsimd.partition_broadcast(recb, rec)
                # no +bv here: softmax weights sum to 1, so bv contributes
                # bv@Wo.T to the residual — folded into bod[0] on the host
                nc.vector.tensor_mul(oT[bp:bp + DK, mt, :], pu[0:DK, :], recb)

            # one-head skew: scores(h+1) are queued on PE before AV(h), so
            # PE streams the next head's QK^T while exp/mask of head h land
            prev = None
            for h in range(H):
                cur = head_scores(h)
                if prev is not None:
                    head_av(h - 1, *prev)
                prev = cur
            head_av(H - 1, *prev)

            # ---- attention out-projection + residual ----
            preload_table(AF.Sqrt)
            wo_sb = load_w(W["wo"], KD, D, "wo", offset=0 if l == 0 else 180)
            bod_row = sm.tile([1, 2 * D], f32, tag="bodrow", bufs=1)
            nc.sync.dma_start(out=bod_row, in_=bass.AP(
                tensor=W["bod"], offset=0, ap=[[2 * D, 1], [1, 2 * D]]))
            bod_sb = wp.tile([P, 2, D], f32, tag="bod")
            for i in range(2):
                nc.gpsimd.partition_broadcast(bod_sb[:, i, :],
                                              bod_row[:, i * D:(i + 1) * D])
            xb_t = []
            for t in range(NT):
                xb = xp.tile([P, D], f32, tag=f"xb{t}", bufs=1)
                nc.vector.tensor_add(xb, x_t[t], bod_sb[:, 0, :])
                xb_t.append(xb)
            pys = []
            for t in range(NT):
                py = ps.tile([P, D], f32, tag="mm")
                for k in range(KD):
                    nc.tensor.matmul(py, oT[:, k, t * P:(t + 1) * P],
                                     wo_sb[:, k, :],
                                     start=(k == 0), stop=(k == KD - 1))
                pys.append(py)

            # ---- FFN ----
            preload_table(AF.Gelu)
            h2T = resid_ln_transpose(pys, xb_t, "h2_", "h2T")
            w1_sb = load_w(W["w1"], KD, DFF, "w1", offset=-250 if l == 0 else 250)
            w2_sb = load_w(W["w2"], KF, D, "w2", offset=-250 if l == 0 else 250)
            gT = act.tile([P, KF, TOK], bf16, tag="gT", bufs=1)
            xb2_t = []
            for t in range(NT):
                xb = xp.tile([P, D], f32, tag=f"xc{t}", bufs=1)
                nc.vector.tensor_add(xb, x_t[t], bod_sb[:, 1, :])
                xb2_t.append(xb)
            for m in range(KF):
                pa = ps.tile([P, TOK], f32, tag="mm")
                for k in range(KD):
                    nc.tensor.matmul(pa, w1_sb[:, k, m * P:(m + 1) * P],
                                     h2T[:, k, :],
                                     start=(k == 0), stop=(k == KD - 1))
                nc.scalar.activation(gT[:, m, :], pa, AF.Gelu,
                                     bias=b1_t[:, m:m + 1], scale=1.0)
            preload_table(AF.Sqrt)
            pzs = []
            for t in range(NT):
                pz = ps.tile([P, D], f32, tag="mm")
                for k in range(KF):
                    nc.tensor.matmul(pz, gT[:, k, t * P:(t + 1) * P],
                                     w2_sb[:, k, :],
                                     start=(k == 0), stop=(k == KF - 1))
                pzs.append(pz)
            pending = (pzs, xb2_t,
                       [xdbg[l, t * P:(t + 1) * P, :] for t in range(NT)]
                       if debug else None)

        # ---- final LN + 8-way AllGather + lm_head ----
        hfT = resid_ln_transpose(pending[0], pending[1], "hf_", "hfT",
                                 dbg=pending[2])
        nc.sync.dma_start(
            out=bass.AP(tensor=af_i, offset=0,
                        ap=[[TOK, P], [P * TOK, KD], [1, TOK]]),
            in_=hfT)
        if sim:
            for rk in range(NC):
                nc.sync.dma_start(out=af_o[rk * D:(rk + 1) * D, :], in_=af_i[:, :])
        else:
            nc.gpsimd.collective_compute(
                "AllGather", Alu.bypass, replica_groups=GROUPS_ALL,
                ins=[af_i.ap().opt()], outs=[af_o.ap().opt()])

        xfp = [act.tile([P, KD, 2 * TOK], bf16, tag=f"xf{c2}", bufs=1,
                        name=f"xfp{c2}") for c2 in range(NC // 2)]
        with tc.high_priority(offset=16):
            for rk in range(NC):
                nc.sync.dma_start(
                    out=xfp[rk // 2][:, :, (rk % 2) * TOK:(rk % 2 + 1) * TOK],
                    in_=bass.AP(tensor=af_o, offset=rk * D * TOK,
                                ap=[[TOK, P], [P * TOK, KD], [1, TOK]]))

        bh_t = sm.tile([P, vt], f32, tag="bh")
        with tc.high_priority(offset=40):
            nc.sync.dma_start(out=bh_t, in_=bass.AP(
                tensor=bhead, offset=0, ap=[[1, P], [P, vt]]))
        NCHUNK = (B * S) // 512
        MC = 13
        for ci, m0 in enumerate(range(0, vt, MC)):
            mn = min(MC, vt - m0)
            woff = min(1500, max(tc.cur_priority - 8, 0))
            with tc.high_priority(offset=woff):
                whc = whp.tile([P, KD, MC * P], bf16, tag="wh", bufs=2)
                nc.sync.dma_start(
                    out=whc[:, :, :mn * P],
                    in_=bass.AP(tensor=whead, offset=m0 * P,
                                ap=[[vt * P, P], [P * vt * P, KD], [1, mn * P]]))
            for mi in range(mn):
                m = m0 + mi
                lo = act.tile([P, B * S], bf16, tag=f"lo{m % 2}", bufs=1)
                for c2 in range(NCHUNK):
                    if (m + c2) % 2 == 0:
                        pl = ps.tile([P, 512], f32, tag="mm")
                    else:
                        pl = pss.tile([P, 512], f32, tag="sc", bufs=3)
                    for k in range(KD):
                        nc.tensor.matmul(
                            pl, whc[:, k, mi * P:(mi + 1) * P],
                            xfp[c2][:, k, :],
                            start=(k == 0), stop=(k == KD - 1))
                    dst = lo[:, c2 * 512:(c2 + 1) * 512]
                    if (m + c2) % 2 == 0:
                        nc.scalar.activation(dst, pl, AF.Identity,
                                             bias=bh_t[:, m:m + 1], scale=1.0)
                    else:
                        nc.vector.tensor_scalar_add(out=dst, in0=pl,
                                                    scalar1=bh_t[:, m:m + 1])
                    if m == vt - 1:
                        # stream the final tile's chunks out as they finish
                        # so the tail is one chunk deep, not a full row
                        nc.sync.dma_start(
                            out=logits_out[m * P:(m + 1) * P,
                                           c2 * 512:(c2 + 1) * 512],
                            in_=lo[:, c2 * 512:(c2 + 1) * 512])
                if m != vt - 1:
                    nc.sync.dma_start(out=logits_out[m * P:(m + 1) * P, :], in_=lo)

    nc.compile()
    return nc


# --------------------------------------------------------------------------
# host side
# --------------------------------------------------------------------------

def host_prep(inputs, n_layers=L, vt=VT_FULL):
    emb = _f32(inputs["embedding"])
    pos = _f32(inputs["pos_embedding"])[0, :S]
    tokens = np.asarray(inputs["tokens"]).astype(np.int64)

    g1 = _f32(inputs["ln1_g"]); b1l = _f32(inputs["ln1_b"])
    g2 = _f32(inputs["ln2_g"]); b2l = _f32(inputs["ln2_b"])
    gf = _f32(inputs["lnf_g"]); bfl = _f32(inputs["lnf_b"])

    shared = {}
    for l in range(n_layers):
        Wq, Wk, Wv, Wo = (_f32(inputs[k][l]) for k in ["Wq", "Wk", "Wv", "Wo"])
        W1, W2 = _f32(inputs["W1"][l]), _f32(inputs["W2"][l])
        bq, bk, bv, bo = (_f32(inputs[k][l]) for k in ["bq", "bk", "bv", "bo"])
        b1, b2 = _f32(inputs["b1"][l]), _f32(inputs["b2"][l])
        shared[f"wq{l}"] = _bf(((Wq * g1[l]) / 8.0).T)
        shared[f"wk{l}"] = _bf((Wk * g1[l]).T)
        shared[f"wv{l}"] = _bf((Wv * g1[l]).T)
        shared[f"wo{l}"] = _bf(Wo.T)
        shared[f"w1{l}"] = _bf((W1 * g2[l]).T)
        shared[f"w2{l}"] = _bf(W2.T)
        bq_e = (bq + Wq @ b1l[l]) / 8.0
        bk_e = bk + Wk @ b1l[l]
        bv_e = bv + Wv @ b1l[l]
        b1_e = b1 + W1 @ b2l[l]
        bvec = np.zeros((P, 28), np.float32)
        bvec[:, 0:4] = bq_e.reshape(4, P).T
        bvec[:, 4:8] = bk_e.reshape(4, P).T
        bvec[:, 8:12] = bv_e.reshape(4, P).T
        bvec[:, 12:28] = b1_e.reshape(16, P).T
        shared[f"bvec{l}"] = bvec
        bod = np.zeros((2, D), np.float32)
        # softmax weights sum to 1, so the v bias reaches x as bv_e @ Wo.T
        bod[0] = bo + bv_e @ Wo.T
        bod[1] = b2
        shared[f"bod{l}"] = bod

    Whead = _f32(inputs["Whead"]); bh = _f32(inputs["bhead"])
    Whead_eff = Whead * gf
    bh_eff = bh + Whead @ bfl

    in_maps = []
    for c in range(NC):
        b, j = c // 4, c % 4
        m = {"x0": np.zeros((TOK, D), np.float32)}
        for g in range(NT):
            t_ids = 512 * g + 4 * np.arange(P) + j
            m["x0"][g * P:(g + 1) * P] = emb[tokens[b, t_ids]] + pos[t_ids]
        mk = np.zeros((4, P, P), np.float32)
        for jp in range(4):
            rk = np.arange(P)[:, None]; rq = np.arange(P)[None, :]
            mk[jp] = (rk <= rq - (1 if jp > j else 0)).astype(np.float32)
        m["masks"] = _bf(mk)
        v0 = sum(VS[:c])
        n = min(VS[c], vt * P)
        wslice = np.zeros((D, vt * P), np.float32)
        bslice = np.zeros((vt * P,), np.float32)
        wslice[:, :n] = Whead_eff.T[:, v0:v0 + n]
        bslice[:n] = bh_eff[v0:v0 + n]
        m["whead"] = _bf(wslice)
        m["bhead"] = _f32(bslice)
        m.update(shared)
        in_maps.append(m)
    return in_maps


def assemble(results, vt=VT_FULL):
    gam = np.arange(NC * TOK)
    cp = gam // TOK; w = gam % TOK
    gp = w // P; rp = w % P
    bb = cp // 4; jj = cp % 4
    t = 512 * gp + 4 * rp + jj
    rows = bb * S + t
    out = np.empty((B * S, V), np.float32)
    for c in range(NC):
        v0 = sum(VS[:c])
        lt = results[c]["logitsT"][:VS[c]]
        out[rows, v0:v0 + VS[c]] = lt.T
    return out.reshape(B, S, V)


_CACHE = {}


def kernel(**inputs):
    key = ("full", L, VT_FULL)
    if key not in _CACHE:
        _CACHE[key] = build(L, VT_FULL, debug=False)
    nc = _CACHE[key]
    in_maps = host_prep(inputs, L, VT_FULL)
    res = run_bass_kernel_spmd(nc, in_maps, list(range(NC)))
    return assemble(res.results, VT_FULL)



# revision 76
# speedup vs baseline: 1.1707x; 1.1707x over previous
"""Trainium2 Bass kernel for a 4-layer GPT-style transformer (B=2, S=1024,
D=512, H=8, DFF=2048, V=50257) on 8 NeuronCores.

Sharding: 2 batch groups x 4-way sequence-parallel (tokens interleaved
stride 4); K/V all-gathered per layer within each group, final hidden
states all-gathered 8-way for a vocab-sharded LM head.  Key optimizations:
bf16 logits, softmax denominator fused into the AV matmul via a ones
column in V, SPLIT K and V AllGathers (scores start as soon as K lands
while V gathers under them), own-block diagonal scores computed straight
from SBUF during the gather flight (the gathered own copy is masked to
zero on the host), two-head-skewed attention pipeline (scores of heads h+1,h+2
queued on PE before AV of head h), deferred residual adds so each tile's
add+LN+transpose chain pipelines against the other tile's matmuls, bias
rows broadcast on-chip via the Pool engine instead of DMAing P copies,
per-rank contiguous KV blocks, capped weight-prefetch priority hoists,
v-bias folded through Wo on the host, streamed final logits writes."""

import numpy as np
import ml_dtypes

import concourse.bass as bass
import concourse.mybir as mybir
import concourse.tile as tile
from concourse import bacc
from concourse.bass_utils import run_bass_kernel_spmd
from concourse.masks import make_identity

AF = mybir.ActivationFunctionType
Alu = mybir.AluOpType
f32 = mybir.dt.float32
bf16 = mybir.dt.bfloat16

V, D, H, DK, DFF, L, B, S = 50257, 512, 8, 64, 2048, 4, 2, 1024
NC, P = 8, 128
EPS = 1e-5
TOK = 256
NT = TOK // P                # 2
KD = D // P                  # 4
KF = DFF // P                # 16
VS_PAD = 6400
VT_FULL = VS_PAD // P        # 50
VS = [6283] * 7 + [V - 7 * 6283]

_bf = lambda a: np.ascontiguousarray(np.asarray(a).astype(ml_dtypes.bfloat16))
_f32 = lambda a: np.ascontiguousarray(np.asarray(a, dtype=np.float32))


def _wload_ap(dram, kdim, n):
    """DRAM [kdim*P, n] viewed as dest-tile order [P, kdim, n]"""
    return bass.AP(tensor=dram, offset=0,
                   ap=[[n, P], [P * n, kdim], [1, n]])


def build(n_layers=L, vt=VT_FULL, debug=False, sim=False):
    nc = bacc.Bacc("TRN2", target_bir_lowering=False, debug=False, num_devices=NC)

    x0_in = nc.dram_tensor("x0", [TOK, D], f32, kind="ExternalInput")
    masks_in = nc.dram_tensor("masks", [5, P, P], bf16, kind="ExternalInput")
    Ws = []
    for l in range(n_layers):
        Ws.append({k: nc.dram_tensor(f"{k}{l}", shp, dt, kind="ExternalInput")
                   for k, shp, dt in [
                       ("wq", [D, D], bf16), ("wk", [D, D], bf16),
                       ("wv", [D, D], bf16), ("wo", [D, D], bf16),
                       ("w1", [D, DFF], bf16), ("w2", [DFF, D], bf16),
                       ("bvec", [P, 28], f32),      # bq|bk|bv|b1 per-partition
                       ("bod", [2, D], f32)]})      # bo | b2 rows (bcast on-chip)
    whead = nc.dram_tensor("whead", [D, vt * P], bf16, kind="ExternalInput")
    bhead = nc.dram_tensor("bhead", [vt * P], f32, kind="ExternalInput")
    logits_out = nc.dram_tensor("logitsT", [vt * P, B * S], bf16, kind="ExternalOutput")
    xdbg = (nc.dram_tensor("xdbg", [n_layers, TOK, D], f32, kind="ExternalOutput")
            if debug else None)

    GROUPS_BATCH = [[0, 1, 2, 3], [4, 5, 6, 7]]
    GROUPS_ALL = [list(range(NC))]
    KSZ = D * TOK                 # KT_own elems per rank
    VSZ = TOK * H * (DK + 1)      # V_own(+ones) elems per rank
    ags = []
    for l in range(n_layers):
        aik = nc.dram_tensor(f"agink{l}", [KSZ], bf16)
        aok = nc.dram_tensor(f"agoutk{l}", [4 * KSZ], bf16)
        aiv = nc.dram_tensor(f"aginv{l}", [VSZ], bf16)
        aov = nc.dram_tensor(f"agoutv{l}", [4 * VSZ], bf16)
        ags.append((aik, aok, aiv, aov))
    af_i = nc.dram_tensor("aginF", [D, TOK], bf16)
    af_o = nc.dram_tensor("agoutF", [NC * D, TOK], bf16, addr_space="Shared")

    import contextlib
    with tile.TileContext(nc) as tc, contextlib.ExitStack() as ctx:
        const = ctx.enter_context(tc.tile_pool(name="const", bufs=1))
        xp = ctx.enter_context(tc.tile_pool(name="xp", bufs=2))
        wp = ctx.enter_context(tc.tile_pool(name="wp", bufs=1))
        whp = ctx.enter_context(tc.tile_pool(name="whp", bufs=1))
        act = ctx.enter_context(tc.tile_pool(name="act", bufs=2))
        atn = ctx.enter_context(tc.tile_pool(name="atn", bufs=2))
        sm = ctx.enter_context(tc.tile_pool(name="sm", bufs=2))
        ps = ctx.enter_context(tc.tile_pool(name="ps", bufs=3, space="PSUM"))
        psu = ctx.enter_context(tc.tile_pool(name="psu", bufs=2, space="PSUM"))
        pss = ctx.enter_context(tc.tile_pool(name="pss", bufs=2, space="PSUM"))

        ident = const.tile([P, P], bf16)
        make_identity(nc, ident)
        ones_col = const.tile([P, 1], bf16)
        nc.vector.memset(ones_col, 1.0)
        eps_t = const.tile([P, 1], f32)
        nc.vector.memset(eps_t, EPS)
        neg1_t = const.tile([P, 1], f32)
        nc.vector.memset(neg1_t, -1.0)
        masks = const.tile([P, 5, P], bf16)
        x_t = [xp.tile([P, D], f32, tag=f"x{t}", name=f"x_{t}") for t in range(NT)]
        with tc.high_priority():
            for t in range(NT):
                nc.sync.dma_start(out=x_t[t], in_=x0_in[t * P:(t + 1) * P, :])
            nc.sync.dma_start(out=masks, in_=bass.AP(
                tensor=masks_in, offset=0, ap=[[P, P], [P * P, 5], [1, P]]))

        def ln_tile(src, tag, t):
            stats = sm.tile([P, 6], f32, tag="stats")
            nc.vector.bn_stats(stats, src)
            mv = sm.tile([P, 2], f32, tag="mv")
            nc.vector.bn_aggr(mv, stats)
            sd = sm.tile([P, 1], f32, tag="sd")
            nc.scalar.activation(sd, mv[:, 1:2], AF.Sqrt, bias=eps_t, scale=1.0)
            nc.vector.reciprocal(sd, sd)
            h = act.tile([P, D], bf16, tag=f"{tag}{t}")
            nc.vector.tensor_scalar(
                out=h, in0=src, scalar1=mv[:, 0:1], scalar2=sd,
                op0=Alu.subtract, op1=Alu.mult)
            return h

        def layernorm(src_tiles, tag):
            return [ln_tile(src_tiles[t], tag, t) for t in range(NT)]

        def transpose_tile(hT, h, t):
            for d in range(KD):
                pt = ps.tile([P, P], bf16, tag="mm", bufs=3)
                nc.tensor.transpose(pt, h[:, d * P:(d + 1) * P], ident)
                if d % 2 == 0:
                    nc.vector.tensor_copy(hT[:, d, t * P:(t + 1) * P], pt)
                else:
                    nc.scalar.copy(hT[:, d, t * P:(t + 1) * P], pt)

        def ln_transpose(src_tiles, tag, ttag, anchors=None):
            hT = act.tile([P, KD, TOK], bf16, tag=ttag)
            for t in range(NT):
                a = anchors[t] if anchors else None
                if a is not None:
                    with tc.high_priority(
                            offset=max(tc.cur_priority - a, 0)):
                        h = ln_tile(src_tiles[t], tag, t)
                else:
                    h = ln_tile(src_tiles[t], tag, t)
                transpose_tile(hT, h, t)
            return hT

        def resid_ln_transpose(pzs, xbs, tag, ttag, dbg=None):
            # deferred residual: tile t's add+LN chain is issued before tile
            # t+1's, so the DVE FIFO runs t0's stats while PE still streams
            # t1's matmuls, and t0's transposes start ~2us earlier
            hT = act.tile([P, KD, TOK], bf16, tag=ttag)
            for t in range(NT):
                xn = xp.tile([P, D], f32, tag=f"x{t}")
                nc.vector.tensor_add(xn, pzs[t], xbs[t])
                x_t[t] = xn
                if dbg is not None:
                    nc.sync.dma_start(out=dbg[t], in_=xn)
                h = ln_tile(xn, tag, t)
                transpose_tile(hT, h, t)
            return hT

        def transpose_own(h_tiles, tag):
            # t-outer so downstream per-t consumers can start after tile 0
            hT = act.tile([P, KD, TOK], bf16, tag=tag)
            for t in range(NT):
                transpose_tile(hT, h_tiles[t], t)
            return hT

        def preload_table(func):
            # no-op: explicit table-load insertion (bacc) already places the
            # LoadActFuncSet before first use; extra dummy activations only
            # added Act-engine work in the measured schedule
            del func

        def load_w(dram, kdim, ndim, tag, offset=180, bufs=1, swdge=False):
            off = min(offset, max(tc.cur_priority - 8, 0))
            with tc.high_priority(offset=off):
                wt = wp.tile([P, kdim, ndim], bf16, tag=tag, name=tag, bufs=bufs)
                # SWDGE (Pool) path skips the serial HWDGE gate; safe only
                # before the first collective is queued on the Pool engine
                eng = nc.gpsimd if swdge else nc.sync
                eng.dma_start(out=wt, in_=_wload_ap(dram, kdim, ndim))
            return wt

        pending = None
        for l in range(n_layers):
            W = Ws[l]
            aik, aok, aiv, aov = ags[l]

            # ---- LN1, transpose ----
            if pending is None:
                hT_own = ln_transpose(x_t, "h1_", "hTown")
            else:
                hT_own = resid_ln_transpose(pending[0], pending[1],
                                            "h1_", "hTown", dbg=pending[2])

            wq_sb = load_w(W["wq"], KD, D, "wq", bufs=2)
            wk_sb = load_w(W["wk"], KD, D, "wk", bufs=2)
            wv_sb = load_w(W["wv"], KD, D, "wv", bufs=2)
            bv_sb = sm.tile([P, 28], f32, tag="bvec")
            nc.sync.dma_start(out=bv_sb, in_=W["bvec"][:, :])
            bq_t = bv_sb[:, 0:4]; bk_t = bv_sb[:, 4:8]
            b1_t = bv_sb[:, 12:28]

            # ---- K, V for OWN tokens first (feeds the AllGather) ----
            # one contiguous [P, KVC] block: K^T cols then V rows, V rows
            # augmented with a per-head ones column (fused softmax denom)
            VW = H * (DK + 1)
            KVC = KD * TOK + NT * VW
            kv_own = atn.tile([P, KVC], bf16, tag="kvown", bufs=1)
            kT_own = kv_own[:, 0:KD * TOK].rearrange(
                "p (k t) -> p k t", k=KD)
            v_own = kv_own[:, KD * TOK:KVC].rearrange(
                "p (t h d) -> p t h d", t=NT, h=H)
            # K-proj per token-tile so tile-0 matmuls start before tile-1's
            # transpose lands
            for t in range(NT):
                for m in range(KD):
                    pk = ps.tile([P, P], f32, tag="mm")
                    for k in range(KD):
                        nc.tensor.matmul(pk, wk_sb[:, k, m * P:(m + 1) * P],
                                         hT_own[:, k, t * P:(t + 1) * P],
                                         start=(k == 0), stop=(k == KD - 1))
                    dst = kT_own[:, m, t * P:(t + 1) * P]
                    if m % 2 == 0:
                        nc.scalar.activation(dst, pk, AF.Identity,
                                             bias=bk_t[:, m:m + 1], scale=1.0)
                    else:
                        nc.vector.tensor_scalar_add(out=dst, in0=pk,
                                                    scalar1=bk_t[:, m:m + 1])

            # ---- AllGather K right away (V's gather launches separately
            # once V is projected, hiding under the K-gated score matmuls) ----
            with tc.high_priority(offset=16):
                nc.sync.dma_start(
                    out=bass.AP(tensor=aik, offset=0, ap=[[KSZ // P, P], [1, KD * TOK]]),
                    in_=kv_own[:, 0:KD * TOK])
                if sim:
                    for jp in range(4):
                        nc.sync.dma_start(out=aok[jp * KSZ:(jp + 1) * KSZ],
                                          in_=aik[:])
                else:
                    nc.gpsimd.collective_compute(
                        "AllGather", Alu.bypass, replica_groups=GROUPS_BATCH,
                        ins=[aik.ap().opt()], outs=[aok.ap().opt()])

            nc.vector.memset(v_own[:, :, :, DK:DK + 1], 1.0)
            for t in range(NT):
                pv = ps.tile([P, H, DK], f32, tag="mm")
                for k in range(KD):
                    nc.tensor.matmul(pv, hT_own[:, k, t * P:(t + 1) * P],
                                     wv_sb[:, k, :], start=(k == 0),
                                     stop=(k == KD - 1))
                if t % 2 == 0:
                    nc.vector.tensor_copy(v_own[:, t, :, 0:DK], pv)
                else:
                    nc.scalar.copy(v_own[:, t, :, 0:DK], pv)

            # ---- AllGather V within the batch group ----
            with tc.high_priority(offset=16):
                nc.sync.dma_start(
                    out=bass.AP(tensor=aiv, offset=0,
                                ap=[[VSZ // P, P], [1, NT * VW]]),
                    in_=kv_own[:, KD * TOK:KVC])
                if sim:
                    for jp in range(4):
                        nc.sync.dma_start(out=aov[jp * VSZ:(jp + 1) * VSZ],
                                          in_=aiv[:])
                else:
                    nc.gpsimd.collective_compute(
                        "AllGather", Alu.bypass, replica_groups=GROUPS_BATCH,
                        ins=[aiv.ap().opt()], outs=[aov.ap().opt()])

            # ---- Q for OWN tokens (overlaps the AllGather flight) ----
            qT = atn.tile([P, KD, TOK], bf16, tag="qT")
            for m in range(KD):
                pq = ps.tile([P, TOK], f32, tag="mm")
                for k in range(KD):
                    nc.tensor.matmul(pq, wq_sb[:, k, m * P:(m + 1) * P],
                                     hT_own[:, k, :],
                                     start=(k == 0), stop=(k == KD - 1))
                nc.scalar.activation(qT[:, m, :], pq, AF.Identity,
                                     bias=bq_t[:, m:m + 1], scale=1.0)
            preload_table(AF.Exp)

            # ---- own-block DIAGONAL scores straight from SBUF while the
            # gathers fly.  Only the two causally-masked diagonal sub-blocks
            # (own k-t0 x q-g0, own k-t1 x q-g1) are precomputed; the
            # gathered pass zeroes exactly those via the zeroed own-jp mask
            # row (pT0's [P:TOK] half and all other blocks flow as before) ----
            pT4s = []
            for h in range(H):
                mt, bp = h // 2, 64 * (h % 2)
                qh = qT[bp:bp + DK, mt, :]
                sc4 = pss.tile([P, 2 * P], f32, tag="sc", bufs=3)
                nc.tensor.matmul(sc4[:, 0:P],
                                 kT_own[bp:bp + DK, mt, 0:P], qh[:, 0:P],
                                 start=True, stop=True, skip_group_check=True)
                nc.tensor.matmul(sc4[:, P:2 * P],
                                 kT_own[bp:bp + DK, mt, P:TOK], qh[:, P:TOK],
                                 start=True, stop=True, skip_group_check=True)
                pT4 = atn.tile([P, 2 * P], bf16, tag=f"pT4_{h}", bufs=1)
                nc.scalar.activation(pT4, sc4, AF.Exp)
                nc.vector.tensor_mul(pT4[:, 0:P], pT4[:, 0:P], masks[:, 4, :])
                nc.vector.tensor_mul(pT4[:, P:2 * P],
                                     pT4[:, P:2 * P], masks[:, 4, :])
                pT4s.append(pT4)

            kvj = [atn.tile([P, KVC], bf16, tag=f"kv{jp}", bufs=1,
                            name=f"kvj{jp}") for jp in range(4)]
            with tc.high_priority(offset=16):
                for jp in range(4):
                    nc.sync.dma_start(
                        out=kvj[jp][:, 0:KD * TOK],
                        in_=bass.AP(tensor=aok, offset=jp * KSZ,
                                    ap=[[KSZ // P, P], [1, KD * TOK]]))
                for jp in range(4):
                    nc.sync.dma_start(
                        out=kvj[jp][:, KD * TOK:KVC],
                        in_=bass.AP(tensor=aov, offset=jp * VSZ,
                                    ap=[[VSZ // P, P], [1, NT * VW]]))

            # ---- attention per head ----
            kTg = [kvj[jp][:, 0:KD * TOK].rearrange("p (k t) -> p k t", k=KD)
                   for jp in range(4)]
            vg = [kvj[jp][:, KD * TOK:KVC].rearrange(
                "p (t h d) -> p t h d", t=NT, h=H) for jp in range(4)]
            oT = atn.tile([P, KD, TOK], bf16, tag="oT")

            def head_scores(h):
                mt, bp = h // 2, 64 * (h % 2)
                kh = lambda col0, n: kTg[col0 // TOK][
                    bp:bp + DK, mt, (col0 % TOK):(col0 % TOK) + n]
                qh = qT[bp:bp + DK, mt, :]
                pT0 = atn.tile([P, 4, TOK], bf16, tag="pT0", bufs=3)
                pT1 = atn.tile([P, 4, P], bf16, tag="pT1", bufs=3)
                for pr in range(2):
                    sc = pss.tile([P, 2 * TOK], f32, tag="sc", bufs=3)
                    for i in range(2):
                        jp = 2 * pr + i
                        nc.tensor.matmul(sc[:, i * TOK:(i + 1) * TOK],
                                         kh(256 * jp, P), qh,
                                         start=True, stop=True,
                                         skip_group_check=True)
                    nc.scalar.activation(pT0[:, 2 * pr:2 * pr + 2, :], sc, AF.Exp)
                    for i in range(2):
                        jp = 2 * pr + i
                        nc.vector.tensor_mul(pT0[:, jp, 0:P], pT0[:, jp, 0:P],
                                             masks[:, jp, :])
                for pr in range(2):
                    sc1 = pss.tile([P, TOK], f32, tag="sc", bufs=3)
                    for i in range(2):
                        jp = 2 * pr + i
                        nc.tensor.matmul(sc1[:, i * P:(i + 1) * P],
                                         kh(256 * jp + P, P), qh[:, P:TOK],
                                         start=True, stop=True,
                                         skip_group_check=True)
                    nc.scalar.activation(pT1[:, 2 * pr:2 * pr + 2, :], sc1, AF.Exp)
                    for i in range(2):
                        jp = 2 * pr + i
                        nc.vector.tensor_mul(pT1[:, jp, :], pT1[:, jp, :],
                                             masks[:, jp, :])
                return pT0, pT1

            def head_av(h, pT0, pT1):
                mt, bp = h // 2, 64 * (h % 2)
                # u^T accumulation; row DK = softmax denominator (ones col)
                pu = psu.tile([DK + 1, TOK], f32, tag="pu")
                vh = lambda i: vg[i // 2][:, i % 2, h, :]
                for jp in range(4):
                    nc.tensor.matmul(pu, vh(2 * jp), pT0[:, jp, :],
                                     start=(jp == 0), stop=False,
                                     skip_group_check=True)
                # own-block diagonal contributions (precomputed pre-gather);
                # narrower-region writes must FOLLOW the full-range start
                nc.tensor.matmul(pu[:, 0:P], v_own[:, 0, h, :],
                                 pT4s[h][:, 0:P],
                                 start=False, stop=False,
                                 skip_group_check=True)
                nc.tensor.matmul(pu[:, P:TOK], v_own[:, 1, h, :],
                                 pT4s[h][:, P:2 * P],
                                 start=False, stop=False,
                                 skip_group_check=True)
                for jp in range(4):
                    nc.tensor.matmul(pu[:, P:TOK], vh(2 * jp + 1), pT1[:, jp, :],
                                     start=False, stop=(jp == 3),
                                     skip_group_check=True)
                rec = sm.tile([1, TOK], f32, tag="rec", bufs=2)
                nc.vector.reciprocal(rec, pu[DK:DK + 1, :])
                recb = sm.tile([DK, TOK], f32, tag="recb", bufs=2)
                nc.gpsimd.partition_broadcast(recb, rec)
                # no +bv here: softmax weights sum to 1, so bv contributes
                # bv@Wo.T to the residual — folded into bod[0] on the host
                nc.vector.tensor_mul(oT[bp:bp + DK, mt, :], pu[0:DK, :], recb)

            # two-head skew: scores(h+1), scores(h+2) are queued on PE before
            # AV(h), so PE streams ahead while exp/mask of head h land
            from collections import deque
            pending_h = deque()
            for h in range(H):
                pending_h.append((h, head_scores(h)))
                if len(pending_h) > 2:
                    hh, p = pending_h.popleft()
                    head_av(hh, *p)
            while pending_h:
                hh, p = pending_h.popleft()
                head_av(hh, *p)

            # ---- attention out-projection + residual ----
            preload_table(AF.Sqrt)
            wo_sb = load_w(W["wo"], KD, D, "wo", offset=0 if l == 0 else 180)
            bod_row = sm.tile([1, 2 * D], f32, tag="bodrow", bufs=1)
            nc.sync.dma_start(out=bod_row, in_=bass.AP(
                tensor=W["bod"], offset=0, ap=[[2 * D, 1], [1, 2 * D]]))
            bod_sb = wp.tile([P, 2, D], f32, tag="bod")
            for i in range(2):
                nc.gpsimd.partition_broadcast(bod_sb[:, i, :],
                                              bod_row[:, i * D:(i + 1) * D])
            xb_t = []
            for t in range(NT):
                xb = xp.tile([P, D], f32, tag=f"xb{t}", bufs=1)
                # Pool: xb has slack until the residual add, and DVE is busy
                # with attention masks/oT scaling in this window
                nc.gpsimd.tensor_add(xb, x_t[t], bod_sb[:, 0, :])
                xb_t.append(xb)
            pys = []
            for t in range(NT):
                py = ps.tile([P, D], f32, tag="mm")
                for k in range(KD):
                    nc.tensor.matmul(py, oT[:, k, t * P:(t + 1) * P],
                                     wo_sb[:, k, :],
                                     start=(k == 0), stop=(k == KD - 1))
                pys.append(py)

            # ---- FFN ----
            preload_table(AF.Gelu)
            h2T = resid_ln_transpose(pys, xb_t, "h2_", "h2T")
            w1_sb = load_w(W["w1"], KD, DFF, "w1", offset=-250 if l == 0 else 250)
            w2_sb = load_w(W["w2"], KF, D, "w2", offset=-250 if l == 0 else 250)
            gT = act.tile([P, KF, TOK], bf16, tag="gT", bufs=1)
            xb2_t = []
            for t in range(NT):
                xb = xp.tile([P, D], f32, tag=f"xc{t}", bufs=1)
                nc.gpsimd.tensor_add(xb, x_t[t], bod_sb[:, 1, :])
                xb2_t.append(xb)
            for m in range(KF):
                pa = ps.tile([P, TOK], f32, tag="mm")
                for k in range(KD):
                    nc.tensor.matmul(pa, w1_sb[:, k, m * P:(m + 1) * P],
                                     h2T[:, k, :],
                                     start=(k == 0), stop=(k == KD - 1))
                nc.scalar.activation(gT[:, m, :], pa, AF.Gelu,
                                     bias=b1_t[:, m:m + 1], scale=1.0)
            preload_table(AF.Sqrt)
            pzs = []
            for t in range(NT):
                pz = ps.tile([P, D], f32, tag="mm")
                for k in range(KF):
                    nc.tensor.matmul(pz, gT[:, k, t * P:(t + 1) * P],
                                     w2_sb[:, k, :],
                                     start=(k == 0), stop=(k == KF - 1))
                pzs.append(pz)
            pending = (pzs, xb2_t,
                       [xdbg[l, t * P:(t + 1) * P, :] for t in range(NT)]
                       if debug else None)

        # ---- final LN + 8-way AllGather + lm_head ----
        hfT = resid_ln_transpose(pending[0], pending[1], "hf_", "hfT",
                                 dbg=pending[2])
        nc.sync.dma_start(
            out=bass.AP(tensor=af_i, offset=0,
                        ap=[[TOK, P], [P * TOK, KD], [1, TOK]]),
            in_=hfT)
        if sim:
            for rk in range(NC):
                nc.sync.dma_start(out=af_o[rk * D:(rk + 1) * D, :], in_=af_i[:, :])
        else:
            nc.gpsimd.collective_compute(
                "AllGather", Alu.bypass, replica_groups=GROUPS_ALL,
                ins=[af_i.ap().opt()], outs=[af_o.ap().opt()])

        xfp = [act.tile([P, KD, 2 * TOK], bf16, tag=f"xf{c2}", bufs=1,
                        name=f"xfp{c2}") for c2 in range(NC // 2)]
        with tc.high_priority(offset=16):
            for rk in range(NC):
                nc.sync.dma_start(
                    out=xfp[rk // 2][:, :, (rk % 2) * TOK:(rk % 2 + 1) * TOK],
                    in_=bass.AP(tensor=af_o, offset=rk * D * TOK,
                                ap=[[TOK, P], [P * TOK, KD], [1, TOK]]))

        bh_t = sm.tile([P, vt], f32, tag="bh")
        with tc.high_priority(offset=40):
            nc.sync.dma_start(out=bh_t, in_=bass.AP(
                tensor=bhead, offset=0, ap=[[1, P], [P, vt]]))
        NCHUNK = (B * S) // 512
        MC = 10
        for ci, m0 in enumerate(range(0, vt, MC)):
            mn = min(MC, vt - m0)
            woff = min(1500, max(tc.cur_priority - 8, 0))
            with tc.high_priority(offset=woff):
                whc = whp.tile([P, KD, MC * P], bf16, tag="wh", bufs=2)
                nc.sync.dma_start(
                    out=whc[:, :, :mn * P],
                    in_=bass.AP(tensor=whead, offset=m0 * P,
                                ap=[[vt * P, P], [P * vt * P, KD], [1, mn * P]]))
            for mi in range(mn):
                m = m0 + mi
                lo = act.tile([P, B * S], bf16, tag=f"lo{m % 2}", bufs=1)
                for c2 in range(NCHUNK):
                    if (m + c2) % 2 == 0:
                        pl = ps.tile([P, 512], f32, tag="mm")
                    else:
                        pl = pss.tile([P, 512], f32, tag="sc", bufs=3)
                    for k in range(KD):
                        nc.tensor.matmul(
                            pl, whc[:, k, mi * P:(mi + 1) * P],
                            xfp[c2][:, k, :],
                            start=(k == 0), stop=(k == KD - 1))
                    dst = lo[:, c2 * 512:(c2 + 1) * 512]
                    if (m + c2) % 2 == 0:
                        nc.scalar.activation(dst, pl, AF.Identity,
                                             bias=bh_t[:, m:m + 1], scale=1.0)
                    else:
                        nc.vector.tensor_scalar_add(out=dst, in0=pl,
                                                    scalar1=bh_t[:, m:m + 1])
                    if m == vt - 1:
                        # stream the final tile's chunks out as they finish
                        # so the tail is one chunk deep, not a full row
                        nc.sync.dma_start(
                            out=logits_out[m * P:(m + 1) * P,
                                           c2 * 512:(c2 + 1) * 512],
                            in_=lo[:, c2 * 512:(c2 + 1) * 512])
                if m != vt - 1:
                    nc.sync.dma_start(out=logits_out[m * P:(m + 1) * P, :], in_=lo)

    nc.compile()
    return nc


# --------------------------------------------------------------------------
# host side
# --------------------------------------------------------------------------

def host_prep(inputs, n_layers=L, vt=VT_FULL):
    emb = _f32(inputs["embedding"])
    pos = _f32(inputs["pos_embedding"])[0, :S]
    tokens = np.asarray(inputs["tokens"]).astype(np.int64)

    g1 = _f32(inputs["ln1_g"]); b1l = _f32(inputs["ln1_b"])
    g2 = _f32(inputs["ln2_g"]); b2l = _f32(inputs["ln2_b"])
    gf = _f32(inputs["lnf_g"]); bfl = _f32(inputs["lnf_b"])

    shared = {}
    for l in range(n_layers):
        Wq, Wk, Wv, Wo = (_f32(inputs[k][l]) for k in ["Wq", "Wk", "Wv", "Wo"])
        W1, W2 = _f32(inputs["W1"][l]), _f32(inputs["W2"][l])
        bq, bk, bv, bo = (_f32(inputs[k][l]) for k in ["bq", "bk", "bv", "bo"])
        b1, b2 = _f32(inputs["b1"][l]), _f32(inputs["b2"][l])
        shared[f"wq{l}"] = _bf(((Wq * g1[l]) / 8.0).T)
        shared[f"wk{l}"] = _bf((Wk * g1[l]).T)
        shared[f"wv{l}"] = _bf((Wv * g1[l]).T)
        shared[f"wo{l}"] = _bf(Wo.T)
        shared[f"w1{l}"] = _bf((W1 * g2[l]).T)
        shared[f"w2{l}"] = _bf(W2.T)
        bq_e = (bq + Wq @ b1l[l]) / 8.0
        bk_e = bk + Wk @ b1l[l]
        bv_e = bv + Wv @ b1l[l]
        b1_e = b1 + W1 @ b2l[l]
        bvec = np.zeros((P, 28), np.float32)
        bvec[:, 0:4] = bq_e.reshape(4, P).T
        bvec[:, 4:8] = bk_e.reshape(4, P).T
        bvec[:, 8:12] = bv_e.reshape(4, P).T
        bvec[:, 12:28] = b1_e.reshape(16, P).T
        shared[f"bvec{l}"] = bvec
        bod = np.zeros((2, D), np.float32)
        # softmax weights sum to 1, so the v bias reaches x as bv_e @ Wo.T
        bod[0] = bo + bv_e @ Wo.T
        bod[1] = b2
        shared[f"bod{l}"] = bod

    Whead = _f32(inputs["Whead"]); bh = _f32(inputs["bhead"])
    Whead_eff = Whead * gf
    bh_eff = bh + Whead @ bfl

    in_maps = []
    for c in range(NC):
        b, j = c // 4, c % 4
        m = {"x0": np.zeros((TOK, D), np.float32)}
        for g in range(NT):
            t_ids = 512 * g + 4 * np.arange(P) + j
            m["x0"][g * P:(g + 1) * P] = emb[tokens[b, t_ids]] + pos[t_ids]
        mk = np.zeros((5, P, P), np.float32)
        rk = np.arange(P)[:, None]; rq = np.arange(P)[None, :]
        for jp in range(4):
            if jp == j:
                continue          # own block comes from SBUF; zero it here
            mk[jp] = (rk <= rq - (1 if jp > j else 0)).astype(np.float32)
        mk[4] = (rk <= rq).astype(np.float32)
        m["masks"] = _bf(mk)
        v0 = sum(VS[:c])
        n = min(VS[c], vt * P)
        wslice = np.zeros((D, vt * P), np.float32)
        bslice = np.zeros((vt * P,), np.float32)
        wslice[:, :n] = Whead_eff.T[:, v0:v0 + n]
        bslice[:n] = bh_eff[v0:v0 + n]
        m["whead"] = _bf(wslice)
        m["bhead"] = _f32(bslice)
        m.update(shared)
        in_maps.append(m)
    return in_maps


def assemble(results, vt=VT_FULL):
    gam = np.arange(NC * TOK)
    cp = gam // TOK; w = gam % TOK
    gp = w // P; rp = w % P
    bb = cp // 4; jj = cp % 4
    t = 512 * gp + 4 * rp + jj
    rows = bb * S + t
    out = np.empty((B * S, V), np.float32)
    for c in range(NC):
        v0 = sum(VS[:c])
        lt = results[c]["logitsT"][:VS[c]]
        out[rows, v0:v0 + VS[c]] = lt.T
    return out.reshape(B, S, V)


_CACHE = {}


def kernel(**inputs):
    key = ("full", L, VT_FULL)
    if key not in _CACHE:
        _CACHE[key] = build(L, VT_FULL, debug=False)
    nc = _CACHE[key]
    in_maps = host_prep(inputs, L, VT_FULL)
    res = run_bass_kernel_spmd(nc, in_maps, list(range(NC)))
    return assemble(res.results, VT_FULL)

